# revision 1
# baseline (speedup 1.0000x reference)
"""Trainium2 Bass kernel for nn_AttentionBlock (GroupNorm + 1x1-conv QKV +
multi-head attention + 1x1-conv proj + residual).

Contract: kernel(**inputs) takes the FULL unsharded inputs (numpy) and
returns the FULL output.  Internally shards data-parallel over batch across
8 NeuronCores (2 samples per core).

Layout strategy (per core, per sample):
  - x viewed [C=512, L=1024], channels on SBUF partitions (4 tiles of 128).
  - GroupNorm stats: free-dim reduce on DVE, cross-partition group combine
    via tiny PE matmuls with 0/1 selector matrices, fan-out back to channels
    via PE, normalize with per-partition scale/bias on DVE.
  - QKV: q,k produced as [c, t] (lhsT = host-pretransposed qkv_w^T);
    v produced directly transposed, v^T [s, c], by using xn as the
    stationary operand.  An extra ones column appended to each v^T chunk
    makes the attention output matmul also emit the softmax normalizer Z.
  - Attention per head: wT[s,t] = sum_c k q (K=64, two heads run
    concurrently via PE row tiling); exp on ScalarE (scale folded in,
    no max-subtraction needed: |scores| <~ 10); a_un[c,t] accumulated over
    s-chunks with lhsT = [v^T | 1]; normalize by 1/Z broadcast.
  - proj: lhsT = host-pretransposed proj_w^T; bias via K=1 ones matmul;
    residual add on DVE; store.
"""

import math
import os

import numpy as np

import concourse.bacc as bacc
import concourse.tile as tile
from concourse import mybir
from concourse.bass_utils import run_bass_kernel_spmd

F32 = mybir.dt.float32
AX = mybir.AxisListType
ALU = mybir.AluOpType
ACT = mybir.ActivationFunctionType

N_CORES = 8
B, C, HH, WW = 16, 512, 32, 32
L = HH * WW            # 1024
BL = B // N_CORES      # batches per core = 2
NH = 8                 # heads
CH = C // NH           # head dim = 64
GROUPS = 32
GS = C // GROUPS       # channels per group = 16
EPS = 1e-5
SCALE2 = 1.0 / math.sqrt(CH)   # combined q*k scale, folded into exp
CT = C // 128          # channel tiles = 4
ST = L // 128          # s-chunks = 8
INV_N = 1.0 / (GS * L)         # 1/16384 for group mean

# matmul dtype for the large matmuls.  Measured on HW: float32r streams at
# ~2 cycles/row; bfloat16 at 1 cycle/row with cheap weight loads.
_MM_CHOICE = os.environ.get("KERNEL_MM_DT", "bf16")
MM_DT = {"bf16": mybir.dt.bfloat16, "f32r": mybir.dt.float32r,
         "f32": F32}[_MM_CHOICE]
# debug bisection: 1=gn, 2=+qkv/vT, 3=+attention, 4=full
STAGE = int(os.environ.get("KERNEL_STAGE", "4"))

LAST_RESULTS = None  # test harness can read exec_time_ns from here


def _mm(ap):
    return ap


def _build_program():
    nc = bacc.Bacc("TRN2", target_bir_lowering=False, debug=False,
                   num_devices=N_CORES)

    x_d = nc.dram_tensor("x", [BL, C, L], F32, kind="ExternalInput").ap()
    out_d = nc.dram_tensor("out", [BL, C, L], F32, kind="ExternalOutput").ap()
    wqT_d = nc.dram_tensor("wqkvT", [C, 3 * C], MM_DT, kind="ExternalInput").ap()
    wpT_d = nc.dram_tensor("wprojT", [C, C], MM_DT, kind="ExternalInput").ap()
    nw_d = nc.dram_tensor("norm_w", [C], F32, kind="ExternalInput").ap()
    nb_d = nc.dram_tensor("norm_b", [C], F32, kind="ExternalInput").ap()
    qb_d = nc.dram_tensor("qkv_b", [3 * C], F32, kind="ExternalInput").ap()
    pb_d = nc.dram_tensor("proj_b", [C], F32, kind="ExternalInput").ap()
    sel_d = nc.dram_tensor("sel", [CT, 128, GROUPS], F32, kind="ExternalInput").ap()
    fan_d = nc.dram_tensor("fan", [CT, GROUPS, 128], F32, kind="ExternalInput").ap()
    ones_d = nc.dram_tensor("ones", [1, 512], F32, kind="ExternalInput").ap()
    oneh_d = nc.dram_tensor("onehot8", [NH, NH * CH], F32, kind="ExternalInput").ap()

    with tile.TileContext(nc) as tc:
        with (
            tc.tile_pool(name="wgt", bufs=1) as wgt,          # persistent weights
            tc.tile_pool(name="xs", bufs=CT) as xs_p,         # raw x tiles
            tc.tile_pool(name="xn", bufs=CT) as xn_p,         # normalized x
            tc.tile_pool(name="qk", bufs=2 * CT) as qk_p,     # q,k [c,t]
            tc.tile_pool(name="vt", bufs=ST) as vt_p,         # v^T (+ones)
            tc.tile_pool(name="ew", bufs=8) as ew_p,          # exp(wT) chunks
            tc.tile_pool(name="apool", bufs=CT) as a_p,       # attention out
            tc.tile_pool(name="zz", bufs=2) as z_p,           # packed 1/Z rows
            tc.tile_pool(name="zb", bufs=2) as zb_p,          # 1/Z broadcast
            tc.tile_pool(name="outs", bufs=2) as out_p,       # residual out
            tc.tile_pool(name="tiny", bufs=8) as tiny,        # gn stats etc.
            tc.tile_pool(name="scr", bufs=2) as scr_p,        # bn stats
            tc.tile_pool(name="pmm", bufs=4, space="PSUM") as pmm,   # 4x 1-bank
            tc.tile_pool(name="pa", bufs=4, space="PSUM") as pa_p,   # 4x 1-bank
        ):
            # ---------------- constants / weights ----------------
            wq = []   # qkv_w^T tiles [128 c', 1536 o]
            wp = []   # proj_w^T tiles [128 c', 512 o]
            sel = []
            fan = []
            nw = []
            nb = []
            for i in range(CT):
                w = wgt.tile([128, 3 * C], MM_DT, tag=f"wq{i}")
                nc.sync.dma_start(w[:], wqT_d[128 * i:128 * (i + 1), :])
                wq.append(w)
                w = wgt.tile([128, C], MM_DT, tag=f"wp{i}")
                nc.sync.dma_start(w[:], wpT_d[128 * i:128 * (i + 1), :])
                wp.append(w)
                s = wgt.tile([128, GROUPS], F32, tag=f"sel{i}")
                nc.sync.dma_start(s[:], sel_d[i, :, :])
                sel.append(s)
                f = wgt.tile([GROUPS, 128], F32, tag=f"fan{i}")
                nc.sync.dma_start(f[:], fan_d[i, :, :])
                fan.append(f)
                t = wgt.tile([128, 1], F32, tag=f"nw{i}")
                nc.sync.dma_start(t[:], nw_d[128 * i:128 * (i + 1)].rearrange("(p a) -> p a", a=1))
                nw.append(t)
                t = wgt.tile([128, 1], F32, tag=f"nb{i}")
                nc.sync.dma_start(t[:], nb_d[128 * i:128 * (i + 1)].rearrange("(p a) -> p a", a=1))
                nb.append(t)
            ones_t = wgt.tile([1, 512], F32, tag="ones")
            nc.sync.dma_start(ones_t[:], ones_d[:, :])
            qbv_t = wgt.tile([1, C], F32, tag="qbv")
            nc.sync.dma_start(qbv_t[:], qb_d[2 * C:3 * C].rearrange("(a b) -> a b", a=1))
            pb_t = wgt.tile([1, C], F32, tag="pb")
            nc.sync.dma_start(pb_t[:], pb_d[:].rearrange("(a b) -> a b", a=1))
            qb_qk = []
            for j in range(2 * CT):
                t = wgt.tile([128, 1], F32, tag=f"qb{j}")
                nc.sync.dma_start(t[:], qb_d[128 * j:128 * (j + 1)].rearrange("(p a) -> p a", a=1))
                qb_qk.append(t)
            oneh_t = wgt.tile([NH, NH * CH], F32, tag="oneh")
            nc.sync.dma_start(oneh_t[:], oneh_d[:, :])
            eps_t = wgt.tile([GROUPS, 1], F32, tag="eps")
            nc.gpsimd.memset(eps_t[:], EPS)
            onecol_t = wgt.tile([128, NH], F32, tag="onecol")
            nc.gpsimd.memset(onecol_t[:], 1.0)

            # ---------------- per-batch body ----------------
            for b in range(BL):
                # ---- load x ----
                xs = []
                for i in range(CT):
                    t = xs_p.tile([128, L], F32, tag="xs")
                    nc.sync.dma_start(t[:], x_d[b, 128 * i:128 * (i + 1), :])
                    xs.append(t)

                # ---- GroupNorm stats (per-channel sum / sumsq) ----
                stats = []
                for i in range(CT):
                    bns = scr_p.tile([128, 2, 6], F32, tag="bns")
                    xv = xs[i][:].rearrange("p (s f) -> p s f", f=512)
                    for sgi in range(2):
                        nc.vector.bn_stats(bns[:, sgi, :], xv[:, sgi, :])
                    mv = tiny.tile([128, 2], F32, tag="mv")
                    nc.vector.bn_aggr(mv[:], bns[:])
                    st = tiny.tile([128, 2], F32, tag="stats")
                    # sum = L*mean ; sumsq = L*(var + mean^2)
                    nc.vector.tensor_scalar_mul(st[:, 0:1], mv[:, 0:1], float(L))
                    m2 = tiny.tile([128, 2], F32, tag="m2")
                    nc.vector.tensor_mul(m2[:, 0:1], mv[:, 0:1], mv[:, 0:1])
                    nc.vector.tensor_add(m2[:, 1:2], mv[:, 1:2], m2[:, 0:1])
                    nc.vector.tensor_scalar_mul(st[:, 1:2], m2[:, 1:2], float(L))
                    stats.append(st)
                pg = pmm.tile([128, 512], F32, tag="mm")
                for i in range(CT):
                    nc.tensor.matmul(pg[0:GROUPS, 0:2], sel[i][:, :], stats[i][:, :],
                                     start=(i == 0), stop=(i == CT - 1))
                # group quantities: mean, E[x2], mean^2, var, std, rstd, mean*rstd
                gq = tiny.tile([GROUPS, 8], F32, tag="gq")
                nc.vector.tensor_scalar_mul(gq[:, 0:1], pg[0:GROUPS, 0:1], INV_N)
                nc.vector.tensor_scalar_mul(gq[:, 1:2], pg[0:GROUPS, 1:2], INV_N)
                nc.vector.tensor_mul(gq[:, 2:3], gq[:, 0:1], gq[:, 0:1])
                nc.vector.tensor_sub(gq[:, 3:4], gq[:, 1:2], gq[:, 2:3])
                nc.scalar.activation(gq[:, 4:5], gq[:, 3:4], ACT.Sqrt,
                                     bias=eps_t[:])
                nc.vector.reciprocal(gq[:, 5:6], gq[:, 4:5])
                nc.vector.tensor_mul(gq[:, 6:7], gq[:, 0:1], gq[:, 5:6])
                # fan out to channels, make per-channel scale/bias
                xn = []
                for i in range(CT):
                    pf = pmm.tile([128, 512], F32, tag="mm")
                    nc.tensor.matmul(pf[0:128, 0:2], fan[i][:, :], gq[:, 5:7],
                                     start=True, stop=True)
                    scb = tiny.tile([128, 3], F32, tag="scb")
                    nc.vector.tensor_mul(scb[:, 0:1], pf[0:128, 0:1], nw[i][:])
                    nc.vector.tensor_mul(scb[:, 1:2], pf[0:128, 1:2], nw[i][:])
                    nc.vector.tensor_sub(scb[:, 2:3], nb[i][:], scb[:, 1:2])
                    t = xn_p.tile([128, L], MM_DT, tag="xn")
                    nc.vector.tensor_scalar(t[:], xs[i][:], scb[:, 0:1], scb[:, 2:3],
                                            op0=ALU.mult, op1=ALU.add)
                    xn.append(t)

                if STAGE == 1:
                    for i in range(CT):
                        nc.sync.dma_start(out_d[b, 128 * i:128 * (i + 1), :],
                                          xn[i][:].bitcast(F32))
                    continue

                # ---- QKV: q,k as [c, t] ----
                qk = []
                for j in range(2 * CT):
                    t = qk_p.tile([128, L], MM_DT, tag="qk")
                    for n in range(2):
                        nsl = slice(512 * n, 512 * (n + 1))
                        pq = pmm.tile([128, 512], F32, tag="mm")
                        for i in range(CT):
                            nc.tensor.matmul(
                                pq[:, :],
                                _mm(wq[i][:, 128 * j:128 * (j + 1)]),
                                _mm(xn[i][:, nsl]),
                                start=(i == 0), stop=(i == CT - 1))
                        nc.vector.tensor_scalar_add(t[:, nsl], pq[:, :],
                                                    qb_qk[j][:])
                    qk.append(t)

                # ---- v^T [s, c] with ones column per head ----
                vt = []
                for m in range(ST):
                    pv = pmm.tile([128, 512], F32, tag="mm")
                    for i in range(CT):
                        nc.tensor.matmul(
                            pv[:, :],
                            _mm(xn[i][:, 128 * m:128 * (m + 1)]),
                            _mm(wq[i][:, 2 * C:3 * C]),
                            start=(i == 0), stop=False)
                    nc.tensor.matmul(pv[:, :], ones_t[0:1, 0:128],
                                     qbv_t[0:1, :], start=False, stop=True)
                    t = vt_p.tile([128, NH * (CH + 1)], MM_DT, tag="vt")
                    dst = t[:].rearrange("p (h x) -> p h x", x=CH + 1)
                    srcv = pv[:, :].rearrange("p (h x) -> p h x", x=CH)
                    nc.vector.tensor_copy(dst[:, :, 0:CH], srcv[:, :, :])
                    nc.vector.tensor_copy(
                        dst[:, :, CH:CH + 1],
                        onecol_t[:].rearrange("p (h a) -> p h a", a=1))
                    vt.append(t)

                if STAGE == 2:
                    for i in range(CT):
                        nc.sync.dma_start(out_d[b, 128 * i:128 * (i + 1), :],
                                          qk[i][:].bitcast(F32))
                    continue

                # ---- attention, head pairs ----
                a_tiles = []
                for hp in range(NH // 2):
                    q_t = qk[hp]
                    k_t = qk[CT + hp]
                    pa_h = {0: [None, None], 1: [None, None]}
                    for hh in range(2):
                        for n in range(2):
                            pa_h[hh][n] = pa_p.tile([CH + 1, 512], F32, tag="pa", name=f"pa{hp}_{hh}_{n}")
                    prev = None
                    for m in range(ST):
                        msl = slice(128 * m, 128 * (m + 1))
                        pw = []
                        for hh, (plo, phi, tp) in enumerate(
                                (((0), (CH), (0, 0)), ((CH), (128), (CH, 0)))):
                            for n in range(2):
                                nsl = slice(512 * n, 512 * (n + 1))
                                p = pmm.tile([128, 512], F32, tag="mm")
                                nc.tensor.matmul(p[:, :],
                                                 _mm(k_t[plo:phi, msl]),
                                                 _mm(q_t[plo:phi, nsl]),
                                                 start=True, stop=True,
                                                 tile_position=tp)
                                pw.append((hh, n, p))
                        ews = []
                        for hh, n, p in pw:
                            e = ew_p.tile([128, 512], MM_DT, tag="ew")
                            nc.scalar.activation(e[:], p[:, :], ACT.Exp,
                                                 scale=SCALE2)
                            ews.append((hh, n, e))
                        if prev is not None:
                            pm, pews = prev
                            for hh, n, e in pews:
                                lhs = vt[pm][:, (2 * hp + hh) * (CH + 1):
                                             (2 * hp + hh + 1) * (CH + 1)]
                                nc.tensor.matmul(pa_h[hh][n][:, :], _mm(lhs),
                                                 _mm(e[:, :]),
                                                 start=(pm == 0), stop=False)
                        prev = (m, ews)
                    pm, pews = prev
                    for hh, n, e in pews:
                        lhs = vt[pm][:, (2 * hp + hh) * (CH + 1):
                                     (2 * hp + hh + 1) * (CH + 1)]
                        nc.tensor.matmul(pa_h[hh][n][:, :], _mm(lhs),
                                         _mm(e[:, :]),
                                         start=False, stop=True)
                    # normalize: 1/Z (DVE) -> broadcast (GpSimd) -> multiply
                    a_t = a_p.tile([128, L], MM_DT, tag="a")
                    for hh in range(2):
                        rz = z_p.tile([1, L], F32, tag="rz", name=f"rz{hp}_{hh}")
                        for n in range(2):
                            nsl = slice(512 * n, 512 * (n + 1))
                            nc.vector.reciprocal(rz[:, nsl],
                                                 pa_h[hh][n][CH:CH + 1, :])
                        rzb = zb_p.tile([CH, L], F32, tag="zb")
                        nc.gpsimd.partition_broadcast(rzb[:], rz[:])
                        rows = slice(CH * hh, CH * (hh + 1))
                        for n in range(2):
                            nsl = slice(512 * n, 512 * (n + 1))
                            nc.vector.tensor_mul(a_t[rows, nsl],
                                                 pa_h[hh][n][0:CH, :],
                                                 rzb[:, nsl])
                    a_tiles.append(a_t)

                if STAGE == 3:
                    for i in range(CT):
                        nc.sync.dma_start(out_d[b, 128 * i:128 * (i + 1), :],
                                          a_tiles[i][:].bitcast(F32))
                    continue

                # ---- proj + residual ----
                for j in range(CT):
                    o_t = out_p.tile([128, L], F32, tag="o")
                    for n in range(2):
                        nsl = slice(512 * n, 512 * (n + 1))
                        pp = pmm.tile([128, 512], F32, tag="mm")
                        for i in range(CT):
                            nc.tensor.matmul(
                                pp[:, :],
                                _mm(wp[i][:, 128 * j:128 * (j + 1)]),
                                _mm(a_tiles[i][:, nsl]),
                                start=(i == 0), stop=False)
                        nc.tensor.matmul(pp[:, :],
                                         pb_t[0:1, 128 * j:128 * (j + 1)],
                                         ones_t[0:1, :], start=False, stop=True)
                        nc.vector.tensor_add(o_t[:, nsl], pp[:, :], xs[j][:, nsl])
                    nc.sync.dma_start(out_d[b, 128 * j:128 * (j + 1), :], o_t[:])

    nc.compile()
    return nc


_prog_cache = {}


def _get_program():
    key = str(MM_DT)
    if key not in _prog_cache:
        _prog_cache[key] = _build_program()
    return _prog_cache[key]


def _host_constants():
    # group selector: sel[i][p, g] = 1 where global group of (tile i, part p)
    # is g;  fan[i][g, p] = same, transposed (for the fan-out matmul lhsT).
    sel = np.zeros((CT, 128, GROUPS), dtype=np.float32)
    fan = np.zeros((CT, GROUPS, 128), dtype=np.float32)
    for i in range(CT):
        for p in range(128):
            g = (128 * i + p) // GS
            sel[i, p, g] = 1.0
            fan[i, g, p] = 1.0
    ones = np.ones((1, 512), dtype=np.float32)
    oneh = np.zeros((NH, NH * CH), dtype=np.float32)
    for h in range(NH):
        oneh[h, CH * h:CH * (h + 1)] = 1.0
    return sel, fan, ones, oneh


def kernel(x, norm_w, norm_b, qkv_w, qkv_b, proj_w, proj_b):
    global LAST_RESULTS
    x = np.ascontiguousarray(np.asarray(x, dtype=np.float32))
    np_mm = mybir.dt.np(MM_DT)
    wqkvT = np.ascontiguousarray(np.asarray(qkv_w, dtype=np.float32).T.astype(np_mm))
    wprojT = np.ascontiguousarray(np.asarray(proj_w, dtype=np.float32).T.astype(np_mm))
    sel, fan, ones, oneh = _host_constants()

    xr = x.reshape(B, C, L)
    nc = _get_program()

    common = {
        "wqkvT": wqkvT,
        "wprojT": wprojT,
        "norm_w": np.ascontiguousarray(norm_w, dtype=np.float32),
        "norm_b": np.ascontiguousarray(norm_b, dtype=np.float32),
        "qkv_b": np.ascontiguousarray(qkv_b, dtype=np.float32),
        "proj_b": np.ascontiguousarray(proj_b, dtype=np.float32),
        "sel": sel,
        "fan": fan,
        "ones": ones,
        "onehot8": oneh,
    }
    in_maps = []
    for c in range(N_CORES):
        m = dict(common)
        m["x"] = np.ascontiguousarray(xr[BL * c:BL * (c + 1)])
        in_maps.append(m)

    trace = os.environ.get("KERNEL_TRACE", "0") == "1"
    kwargs = {}
    if trace:
        kwargs = dict(trace=True, trace_cores=[0])
    res = run_bass_kernel_spmd(nc, in_maps, core_ids=list(range(N_CORES)),
                               **kwargs)
    LAST_RESULTS = res
    out = np.concatenate([res.results[c]["out"] for c in range(N_CORES)], axis=0)
    return out.reshape(B, C, HH, WW)



# revision 10
# speedup vs baseline: 1.6037x; 1.6037x over previous
"""Trainium2 Bass kernel for nn_AttentionBlock (GroupNorm + 1x1-conv QKV +
multi-head attention + 1x1-conv proj + residual).

Contract: kernel(**inputs) takes the FULL unsharded inputs (numpy) and
returns the FULL output.  Internally shards data-parallel over batch across
8 NeuronCores (2 samples per core).

v2 design (vs v1 baseline, 558-619us):
  - Attention runs one head at a time, software-pipelined so ScalarE's exp
    stream is saturated: QK scores land as [128 s, 1024 t] f32 in a 2-bank
    PSUM tile (2 matmuls, N=512 each), ONE exp per (head, s-chunk) over
    [128, 1024] (amortizes the ~350-cycle ACT fixed cost), double-buffered
    score tiles so QK(m+1) overlaps exp(m).
  - AV accumulates [v^T | 1] @ e into a [65, 1024] PSUM tile per head; the
    extra ones-column emits the softmax normalizer Z as row 64.
  - 1/Z via reciprocal_approx_fast (single-pass custom DVE op, ~5x faster
    than iterative reciprocal); broadcast via GpSimd; one [64, 1024]
    normalize-multiply per head.
  - v-bias and proj-bias fold into a single host-computed per-channel
    constant b_eff = proj_w @ v_bias + proj_b (softmax rows sum to 1), added
    into the residual input — kills all K=1 bias matmuls (57us of PE time).
  - QKV/proj evacuations as single [128, 1024] DVE ops over 2-bank PSUM
    accumulator pairs.
  - v^T stored in two persistent [128, 8*8*65] tiles (one per sample parity)
    with the ones-columns memset once, evacuated two s-chunks per DVE op.
"""

import math
import os

import numpy as np

import concourse.bacc as bacc
import concourse.tile as tile
from concourse import mybir
from concourse.bass_utils import run_bass_kernel_spmd

F32 = mybir.dt.float32
AX = mybir.AxisListType
ALU = mybir.AluOpType
ACT = mybir.ActivationFunctionType

N_CORES = 8
B, C, HH, WW = 16, 512, 32, 32
L = HH * WW            # 1024
BL = B // N_CORES      # batches per core = 2
NH = 8                 # heads
CH = C // NH           # head dim = 64
GROUPS = 32
GS = C // GROUPS       # channels per group = 16
EPS = 1e-5
SCALE2 = 1.0 / math.sqrt(CH)   # combined q*k scale, folded into exp
CT = C // 128          # channel tiles = 4
ST = L // 128          # s-chunks = 8
INV_N = 1.0 / (GS * L)         # 1/16384 for group mean

MM_DT = mybir.dt.bfloat16
# debug bisection: 1=gn, 2=+qkv, 4=full
STAGE = int(os.environ.get("KERNEL_STAGE", "4"))

LAST_RESULTS = None  # test harness can read exec_time_ns from here


def _build_program():
    nc = bacc.Bacc("TRN2", target_bir_lowering=False, debug=False,
                   num_devices=N_CORES)

    x_d = nc.dram_tensor("x", [BL, C, L], F32, kind="ExternalInput").ap()
    out_d = nc.dram_tensor("out", [BL, C, L], F32, kind="ExternalOutput").ap()
    wqT_d = nc.dram_tensor("wqkvT", [C, 3 * C], MM_DT, kind="ExternalInput").ap()
    wpT_d = nc.dram_tensor("wprojT", [C, C], MM_DT, kind="ExternalInput").ap()
    nw_d = nc.dram_tensor("norm_w", [C], F32, kind="ExternalInput").ap()
    nb_d = nc.dram_tensor("norm_b", [C], F32, kind="ExternalInput").ap()
    qb_d = nc.dram_tensor("qkv_b", [3 * C], F32, kind="ExternalInput").ap()
    beff_d = nc.dram_tensor("b_eff", [C], F32, kind="ExternalInput").ap()
    sel_d = nc.dram_tensor("sel", [CT, 128, GROUPS], F32, kind="ExternalInput").ap()
    fan_d = nc.dram_tensor("fan", [CT, GROUPS, 128], F32, kind="ExternalInput").ap()

    VW = NH * (CH + 1)         # 520: per-s-chunk v^T row width (8 heads x 65)

    with tile.TileContext(nc) as tc:
        with (
            tc.tile_pool(name="wgt", bufs=1) as wgt,          # persistent
            tc.tile_pool(name="xs", bufs=2 * CT) as xs_p,     # raw x tiles
            tc.tile_pool(name="xn", bufs=2 * CT) as xn_p,     # normalized x
            tc.tile_pool(name="qk", bufs=4 * CT) as qk_p,     # q,k [c,t] both samples
            tc.tile_pool(name="ew", bufs=4) as ew_p,          # exp(wT) chunks
            tc.tile_pool(name="apool", bufs=2 * CT) as a_p,   # attention out
            tc.tile_pool(name="zz", bufs=2) as z_p,           # 1/Z rows
            tc.tile_pool(name="zb", bufs=2) as zb_p,          # 1/Z broadcast
            tc.tile_pool(name="outs", bufs=2) as out_p,       # residual out
            tc.tile_pool(name="tiny", bufs=8) as tiny,        # gn stats etc.
            tc.tile_pool(name="scr", bufs=2) as scr_p,        # bn stats
            tc.tile_pool(name="ps", bufs=2, space="PSUM") as ps_p,  # shared
        ):
            # ---------------- constants / weights ----------------
            wq = []   # qkv_w^T tiles [128 c', 1536 o]
            wp = []   # proj_w^T tiles [128 c', 512 o]
            sel = []
            fan = []
            nw = []
            nb = []
            beff = []
            for i in range(CT):
                w = wgt.tile([128, 3 * C], MM_DT, tag=f"wq{i}")
                nc.sync.dma_start(w[:], wqT_d[128 * i:128 * (i + 1), :])
                wq.append(w)
                w = wgt.tile([128, C], MM_DT, tag=f"wp{i}")
                nc.sync.dma_start(w[:], wpT_d[128 * i:128 * (i + 1), :])
                wp.append(w)
                s = wgt.tile([128, GROUPS], F32, tag=f"sel{i}")
                nc.sync.dma_start(s[:], sel_d[i, :, :])
                sel.append(s)
                f = wgt.tile([GROUPS, 128], F32, tag=f"fan{i}")
                nc.sync.dma_start(f[:], fan_d[i, :, :])
                fan.append(f)
                t = wgt.tile([128, 1], F32, tag=f"nw{i}")
                nc.sync.dma_start(t[:], nw_d[128 * i:128 * (i + 1)].rearrange("(p a) -> p a", a=1))
                nw.append(t)
                t = wgt.tile([128, 1], F32, tag=f"nb{i}")
                nc.sync.dma_start(t[:], nb_d[128 * i:128 * (i + 1)].rearrange("(p a) -> p a", a=1))
                nb.append(t)
                t = wgt.tile([128, 1], F32, tag=f"beff{i}")
                nc.sync.dma_start(t[:], beff_d[128 * i:128 * (i + 1)].rearrange("(p a) -> p a", a=1))
                beff.append(t)
            qb_qk = []
            for j in range(2 * CT):
                t = wgt.tile([128, 1], F32, tag=f"qb{j}")
                nc.sync.dma_start(t[:], qb_d[128 * j:128 * (j + 1)].rearrange("(p a) -> p a", a=1))
                qb_qk.append(t)
            eps_t = wgt.tile([GROUPS, 1], F32, tag="eps")
            nc.gpsimd.memset(eps_t[:], EPS)
            # persistent v^T stores, one per sample parity; ones-columns are
            # memset once and never overwritten (evacs write cols 0:64 of
            # each 65-block only)
            vt_all = []
            for p in range(BL):
                v = wgt.tile([128, ST * VW], MM_DT, tag=f"vt{p}")
                nc.gpsimd.memset(v[:], 1.0)
                vt_all.append(v)

            # ---------------- per-batch body ----------------
            for b in range(BL):
                # ---- load x ----
                xs = []
                for i in range(CT):
                    t = xs_p.tile([128, L], F32, tag="xs")
                    nc.sync.dma_start(t[:], x_d[b, 128 * i:128 * (i + 1), :])
                    xs.append(t)

                # ---- GroupNorm stats (per-channel sum / sumsq) ----
                stats = []
                for i in range(CT):
                    bns = scr_p.tile([128, 2, 6], F32, tag="bns")
                    xv = xs[i][:].rearrange("p (s f) -> p s f", f=512)
                    for sgi in range(2):
                        nc.vector.bn_stats(bns[:, sgi, :], xv[:, sgi, :])
                    mv = tiny.tile([128, 2], F32, tag="mv")
                    nc.vector.bn_aggr(mv[:], bns[:])
                    st = tiny.tile([128, 2], F32, tag="stats")
                    # sum = L*mean ; sumsq = L*(var + mean^2)
                    nc.vector.tensor_scalar_mul(st[:, 0:1], mv[:, 0:1], float(L))
                    m2 = tiny.tile([128, 2], F32, tag="m2")
                    nc.vector.tensor_mul(m2[:, 0:1], mv[:, 0:1], mv[:, 0:1])
                    nc.vector.tensor_add(m2[:, 1:2], mv[:, 1:2], m2[:, 0:1])
                    nc.vector.tensor_scalar_mul(st[:, 1:2], m2[:, 1:2], float(L))
                    stats.append(st)
                pg = ps_p.tile([128, 1024], F32, tag="mm", name="pg")
                for i in range(CT):
                    nc.tensor.matmul(pg[0:GROUPS, 0:2], sel[i][:, :], stats[i][:, :],
                                     start=(i == 0), stop=(i == CT - 1))
                # group quantities: mean, E[x2], mean^2, var, std, rstd, mean*rstd
                gq = tiny.tile([GROUPS, 8], F32, tag="gq")
                nc.vector.tensor_scalar_mul(gq[:, 0:1], pg[0:GROUPS, 0:1], INV_N)
                nc.vector.tensor_scalar_mul(gq[:, 1:2], pg[0:GROUPS, 1:2], INV_N)
                nc.vector.tensor_mul(gq[:, 2:3], gq[:, 0:1], gq[:, 0:1])
                nc.vector.tensor_sub(gq[:, 3:4], gq[:, 1:2], gq[:, 2:3])
                nc.scalar.activation(gq[:, 4:5], gq[:, 3:4], ACT.Sqrt,
                                     bias=eps_t[:])
                nc.vector.reciprocal(gq[:, 5:6], gq[:, 4:5])
                nc.vector.tensor_mul(gq[:, 6:7], gq[:, 0:1], gq[:, 5:6])
                # fan out to channels, make per-channel scale/bias
                xn = []
                for i in range(CT):
                    pf = ps_p.tile([128, 1024], F32, tag="mm", name="pf")
                    nc.tensor.matmul(pf[0:128, 0:2], fan[i][:, :], gq[:, 5:7],
                                     start=True, stop=True)
                    scb = tiny.tile([128, 3], F32, tag="scb")
                    nc.vector.tensor_mul(scb[:, 0:1], pf[0:128, 0:1], nw[i][:])
                    nc.vector.tensor_mul(scb[:, 1:2], pf[0:128, 1:2], nw[i][:])
                    nc.vector.tensor_sub(scb[:, 2:3], nb[i][:], scb[:, 1:2])
                    t = xn_p.tile([128, L], MM_DT, tag="xn")
                    nc.vector.tensor_scalar(t[:], xs[i][:], scb[:, 0:1], scb[:, 2:3],
                                            op0=ALU.mult, op1=ALU.add)
                    xn.append(t)
                    # xs now only feeds the residual: fold b_eff in
                    nc.vector.tensor_scalar_add(xs[i][:], xs[i][:], beff[i][:])

                if STAGE == 1:
                    for i in range(CT):
                        nc.sync.dma_start(out_d[b, 128 * i:128 * (i + 1), 0:512],
                                          xn[i][:].bitcast(F32))
                    continue

                # ---- QKV: q,k as [c, t], one 2-bank accumulator per j ----
                qk = []
                for j in range(2 * CT):
                    pq = ps_p.tile([128, 1024], F32, tag="mm", name="pq")
                    for n in range(2):
                        nsl = slice(512 * n, 512 * (n + 1))
                        for i in range(CT):
                            nc.tensor.matmul(
                                pq[:, nsl],
                                wq[i][:, 128 * j:128 * (j + 1)],
                                xn[i][:, nsl],
                                start=(i == 0), stop=(i == CT - 1))
                    t = qk_p.tile([128, L], MM_DT, tag="qk")
                    nc.vector.tensor_scalar_add(t[:], pq[:, :], qb_qk[j][:])
                    qk.append(t)

                # ---- v^T [s, c] into persistent vt tile (2 chunks/evac) ----
                vt = vt_all[b % BL]
                vtv = vt[:].rearrange("p (m h x) -> p m h x", h=NH, x=CH + 1)
                for mp in range(ST // 2):
                    pv = ps_p.tile([128, 1024], F32, tag="mm", name="pv")
                    for half in range(2):
                        m = 2 * mp + half
                        nsl = slice(512 * half, 512 * (half + 1))
                        for i in range(CT):
                            nc.tensor.matmul(
                                pv[:, nsl],
                                xn[i][:, 128 * m:128 * (m + 1)],
                                wq[i][:, 2 * C:3 * C],
                                start=(i == 0), stop=(i == CT - 1))
                    src = pv[:, :].rearrange("p (m h x) -> p m h x", h=NH, x=CH)
                    nc.vector.tensor_copy(vtv[:, 2 * mp:2 * mp + 2, :, 0:CH], src)

                if STAGE == 2:
                    for i in range(CT):
                        nc.sync.dma_start(out_d[b, 128 * i:128 * (i + 1), 0:512],
                                          qk[i][:].bitcast(F32))
                    continue

                # ---- attention, one head at a time, pipelined ----
                a_tiles = [None] * CT
                for h in range(NH):
                    jq = h // 2
                    rq = slice(64 * (h % 2), 64 * (h % 2) + 64)
                    tp = (64 * (h % 2), 0)
                    q_t = qk[jq]
                    k_t = qk[CT + jq]
                    pa_t = ps_p.tile([CH + 1, 1024], F32, tag="pa",
                                     name=f"pa{h}")
                    prev = None
                    for m in range(ST):
                        msl = slice(128 * m, 128 * (m + 1))
                        kq_t = ps_p.tile([128, 1024], F32, tag="mm",
                                         name=f"kq{h}_{m}")
                        for n in range(2):
                            nsl = slice(512 * n, 512 * (n + 1))
                            nc.tensor.matmul(kq_t[:, nsl],
                                             k_t[rq, msl],
                                             q_t[rq, nsl],
                                             start=True, stop=True,
                                             tile_position=tp)
                        e_t = ew_p.tile([128, L], MM_DT, tag="ew")
                        nc.scalar.activation(e_t[:], kq_t[:, :], ACT.Exp,
                                             scale=SCALE2)
                        if prev is not None:
                            pm, pe = prev
                            lhs = vt[:, (pm * NH + h) * (CH + 1):
                                     (pm * NH + h + 1) * (CH + 1)]
                            for n in range(2):
                                nsl = slice(512 * n, 512 * (n + 1))
                                nc.tensor.matmul(pa_t[:, nsl], lhs, pe[:, nsl],
                                                 start=(pm == 0), stop=False)
                        prev = (m, e_t)
                    pm, pe = prev
                    lhs = vt[:, (pm * NH + h) * (CH + 1):
                             (pm * NH + h + 1) * (CH + 1)]
                    for n in range(2):
                        nsl = slice(512 * n, 512 * (n + 1))
                        nc.tensor.matmul(pa_t[:, nsl], lhs, pe[:, nsl],
                                         start=False, stop=True)
                    # normalize: 1/Z (fast approx) -> broadcast -> multiply
                    # (reciprocal_approx_fast reads PSUM wrongly in this
                    # context — hop through SBUF partition 0 first)
                    zt = z_p.tile([1, L], F32, tag="zt", name=f"zt{h}")
                    nc.vector.tensor_copy(zt[:], pa_t[CH:CH + 1, :])
                    rz = z_p.tile([1, L], F32, tag="rz", name=f"rz{h}")
                    nc.vector.reciprocal_approx_fast(rz[:], zt[:])
                    rzb = zb_p.tile([CH, L], F32, tag="zb")
                    nc.gpsimd.partition_broadcast(rzb[:], rz[:])
                    if h % 2 == 0:
                        a_tiles[h // 2] = a_p.tile([128, L], MM_DT, tag="a",
                                                   name=f"a{h // 2}")
                    rows = slice(CH * (h % 2), CH * (h % 2) + CH)
                    nc.vector.tensor_mul(a_tiles[h // 2][rows, :],
                                         pa_t[0:CH, :], rzb[:])
                    if STAGE == 5:
                        # dump Z row and rz for inspection
                        zd = z_p.tile([1, L], F32, tag="zd", name=f"zd{h}")
                        nc.vector.tensor_copy(zd[:], pa_t[CH:CH + 1, :])
                        nc.sync.dma_start(out_d[b, h:h + 1, :], zd[:])
                        nc.sync.dma_start(out_d[b, NH + h:NH + h + 1, :], rz[:])

                if STAGE == 5:
                    continue

                if STAGE == 3:
                    for i in range(CT):
                        nc.sync.dma_start(out_d[b, 128 * i:128 * (i + 1), 0:512],
                                          a_tiles[i][:].bitcast(F32))
                    continue

                # ---- proj + residual ----
                for j in range(CT):
                    pp = ps_p.tile([128, 1024], F32, tag="mm", name="pp")
                    for n in range(2):
                        nsl = slice(512 * n, 512 * (n + 1))
                        for i in range(CT):
                            nc.tensor.matmul(
                                pp[:, nsl],
                                wp[i][:, 128 * j:128 * (j + 1)],
                                a_tiles[i][:, nsl],
                                start=(i == 0), stop=(i == CT - 1))
                    o_t = out_p.tile([128, L], F32, tag="o")
                    nc.vector.tensor_add(o_t[:], pp[:, :], xs[j][:])
                    nc.sync.dma_start(out_d[b, 128 * j:128 * (j + 1), :], o_t[:])

    nc.compile()
    return nc


_prog_cache = {}


def _get_program():
    if "prog" not in _prog_cache:
        _prog_cache["prog"] = _build_program()
    return _prog_cache["prog"]


def _host_constants():
    # group selector: sel[i][p, g] = 1 where global group of (tile i, part p)
    # is g;  fan[i][g, p] = same, transposed (for the fan-out matmul lhsT).
    sel = np.zeros((CT, 128, GROUPS), dtype=np.float32)
    fan = np.zeros((CT, GROUPS, 128), dtype=np.float32)
    for i in range(CT):
        for p in range(128):
            g = (128 * i + p) // GS
            sel[i, p, g] = 1.0
            fan[i, g, p] = 1.0
    return sel, fan


def kernel(x, norm_w, norm_b, qkv_w, qkv_b, proj_w, proj_b):
    global LAST_RESULTS
    x = np.ascontiguousarray(np.asarray(x, dtype=np.float32))
    np_mm = mybir.dt.np(MM_DT)
    qkv_w = np.asarray(qkv_w, dtype=np.float32)
    proj_w = np.asarray(proj_w, dtype=np.float32)
    qkv_b = np.ascontiguousarray(np.asarray(qkv_b, dtype=np.float32))
    proj_b = np.ascontiguousarray(np.asarray(proj_b, dtype=np.float32))
    wqkvT = np.ascontiguousarray(qkv_w.T.astype(np_mm))
    wprojT = np.ascontiguousarray(proj_w.T.astype(np_mm))
    # softmax rows sum to 1, so the v-bias contributes exactly
    # proj_w @ v_bias to the proj output; fold it plus proj_b into one
    # per-channel constant added at the residual.
    b_eff = np.ascontiguousarray(
        proj_w @ qkv_b[2 * C:3 * C] + proj_b).astype(np.float32)
    sel, fan = _host_constants()

    xr = x.reshape(B, C, L)
    nc = _get_program()

    common = {
        "wqkvT": wqkvT,
        "wprojT": wprojT,
        "norm_w": np.ascontiguousarray(norm_w, dtype=np.float32),
        "norm_b": np.ascontiguousarray(norm_b, dtype=np.float32),
        "qkv_b": qkv_b,
        "b_eff": b_eff,
        "sel": sel,
        "fan": fan,
    }
    in_maps = []
    for c in range(N_CORES):
        m = dict(common)
        m["x"] = np.ascontiguousarray(xr[BL * c:BL * (c + 1)])
        in_maps.append(m)

    trace = os.environ.get("KERNEL_TRACE", "0") == "1"
    kwargs = {}
    if trace:
        kwargs = dict(trace=True, trace_cores=[0])
    res = run_bass_kernel_spmd(nc, in_maps, core_ids=list(range(N_CORES)),
                               **kwargs)
    LAST_RESULTS = res
    out = np.concatenate([res.results[c]["out"] for c in range(N_CORES)], axis=0)
    return out.reshape(B, C, HH, WW)


# revision 11
# speedup vs baseline: 1.6195x; 1.0099x over previous
"""Trainium2 Bass kernel for nn_AttentionBlock (GroupNorm + 1x1-conv QKV +
multi-head attention + 1x1-conv proj + residual).

Contract: kernel(**inputs) takes the FULL unsharded inputs (numpy) and
returns the FULL output.  Internally shards data-parallel over batch across
8 NeuronCores (2 samples per core).

v3 design notes (baseline 558-619us, v2 348us):
  - Attention runs one head at a time, software-pipelined so ScalarE's exp
    stream is saturated: QK scores land as [128 s, 1024 t] f32 in a 2-bank
    PSUM tile (one [128,1024] exp per (head, s-chunk) amortizes the
    ~350-cycle ACT fixed cost); score tiles round-robin through 2 buffers so
    QK(m+1) overlaps exp(m).
  - AV accumulates [v^T | 1] @ e into a [65, 1024] PSUM tile per head; the
    extra ones-column emits the softmax normalizer Z as row 64.
  - 1/Z via reciprocal_approx_fast (single-pass custom DVE op) after a hop
    through SBUF; broadcast via GpSimd; one [64, 1024] multiply per head.
  - v-bias and proj-bias fold into b_eff = proj_w @ v_bias + proj_b
    (softmax rows sum to 1), added into the residual input on DVE.
  - HAM throttle management (the big v3 win): the PE clock halves (K=4/8)
    after any >3.4us PE-idle window and, once cold, an exp-gated attention
    loop never re-warms.  So: sample 1's GN/V/QKV phases are emitted
    INTERLEAVED into sample 0's attention (the Scalar-bound stretch has PE
    headroom), the V phase precedes QKV (its PSUM evacuations otherwise
    stall attention startup), and x/GN-constant DMAs precede the big weight
    DMAs so compute starts early.
"""

import math
import os

import numpy as np

import concourse.bacc as bacc
import concourse.tile as tile
from concourse import mybir
from concourse.bass_utils import run_bass_kernel_spmd

F32 = mybir.dt.float32
AX = mybir.AxisListType
ALU = mybir.AluOpType
ACT = mybir.ActivationFunctionType

N_CORES = 8
B, C, HH, WW = 16, 512, 32, 32
L = HH * WW            # 1024
BL = B // N_CORES      # batches per core = 2
NH = 8                 # heads
CH = C // NH           # head dim = 64
GROUPS = 32
GS = C // GROUPS       # channels per group = 16
EPS = 1e-5
SCALE2 = 1.0 / math.sqrt(CH)   # combined q*k scale, folded into exp
CT = C // 128          # channel tiles = 4
ST = L // 128          # s-chunks = 8
INV_N = 1.0 / (GS * L)         # 1/16384 for group mean

MM_DT = mybir.dt.bfloat16
# debug bisection: 1=gn, 2=+qkv, 4=full, 5=dump Z
STAGE = int(os.environ.get("KERNEL_STAGE", "4"))

LAST_RESULTS = None  # test harness can read exec_time_ns from here


def _build_program():
    nc = bacc.Bacc("TRN2", target_bir_lowering=False, debug=False,
                   num_devices=N_CORES)

    x_d = nc.dram_tensor("x", [BL, C, L], F32, kind="ExternalInput").ap()
    out_d = nc.dram_tensor("out", [BL, C, L], F32, kind="ExternalOutput").ap()
    wqT_d = nc.dram_tensor("wqkvT", [C, 3 * C], MM_DT, kind="ExternalInput").ap()
    wpT_d = nc.dram_tensor("wprojT", [C, C], MM_DT, kind="ExternalInput").ap()
    nw_d = nc.dram_tensor("norm_w", [C], F32, kind="ExternalInput").ap()
    nb_d = nc.dram_tensor("norm_b", [C], F32, kind="ExternalInput").ap()
    qb_d = nc.dram_tensor("qkv_b", [3 * C], F32, kind="ExternalInput").ap()
    beff_d = nc.dram_tensor("b_eff", [C], F32, kind="ExternalInput").ap()
    sel_d = nc.dram_tensor("sel", [CT, 128, GROUPS], F32, kind="ExternalInput").ap()
    fan_d = nc.dram_tensor("fan", [CT, GROUPS, 128], F32, kind="ExternalInput").ap()

    VW = NH * (CH + 1)         # 520: per-s-chunk v^T row width (8 heads x 65)

    with tile.TileContext(nc) as tc:
        with (
            tc.tile_pool(name="wgt", bufs=1) as wgt,          # persistent
            tc.tile_pool(name="xs", bufs=2 * CT) as xs_p,     # raw x tiles
            tc.tile_pool(name="xn", bufs=2 * CT) as xn_p,     # normalized x
            tc.tile_pool(name="qk", bufs=4 * CT) as qk_p,     # q,k both samples
            tc.tile_pool(name="ew", bufs=4) as ew_p,          # exp(wT) chunks
            tc.tile_pool(name="apool", bufs=2 * CT) as a_p,   # attention out
            tc.tile_pool(name="zz", bufs=2) as z_p,           # Z rows / 1/Z
            tc.tile_pool(name="zb", bufs=2) as zb_p,          # 1/Z broadcast
            tc.tile_pool(name="outs", bufs=2) as out_p,       # residual out
            tc.tile_pool(name="tiny", bufs=8) as tiny,        # gn stats etc.
            tc.tile_pool(name="scr", bufs=2) as scr_p,        # bn stats
            tc.tile_pool(name="ps", bufs=2, space="PSUM") as ps_p,  # shared
        ):
            # ---------------- x(b=0) + GN constants first ----------------
            st8 = {}   # per-sample dicts of live tiles
            for b in range(BL):
                st8[b] = {}

            def load_x(b):
                xs = []
                for i in range(CT):
                    t = xs_p.tile([128, L], F32, tag="xs", name=f"xs{b}_{i}")
                    nc.sync.dma_start(t[:], x_d[b, 128 * i:128 * (i + 1), :])
                    xs.append(t)
                st8[b]["xs"] = xs

            load_x(0)

            sel = []
            fan = []
            nw = []
            nb = []
            beff = []
            for i in range(CT):
                s = wgt.tile([128, GROUPS], F32, tag=f"sel{i}")
                nc.sync.dma_start(s[:], sel_d[i, :, :])
                sel.append(s)
                f = wgt.tile([GROUPS, 128], F32, tag=f"fan{i}")
                nc.sync.dma_start(f[:], fan_d[i, :, :])
                fan.append(f)
                t = wgt.tile([128, 1], F32, tag=f"nw{i}")
                nc.sync.dma_start(t[:], nw_d[128 * i:128 * (i + 1)].rearrange("(p a) -> p a", a=1))
                nw.append(t)
                t = wgt.tile([128, 1], F32, tag=f"nb{i}")
                nc.sync.dma_start(t[:], nb_d[128 * i:128 * (i + 1)].rearrange("(p a) -> p a", a=1))
                nb.append(t)
                t = wgt.tile([128, 1], F32, tag=f"beff{i}")
                nc.sync.dma_start(t[:], beff_d[128 * i:128 * (i + 1)].rearrange("(p a) -> p a", a=1))
                beff.append(t)
            eps_t = wgt.tile([GROUPS, 1], F32, tag="eps")
            nc.gpsimd.memset(eps_t[:], EPS)

            # ---------------- big weights ----------------
            wq = []   # qkv_w^T tiles [128 c', 1536 o]
            wp = []   # proj_w^T tiles [128 c', 512 o]
            for i in range(CT):
                w = wgt.tile([128, 3 * C], MM_DT, tag=f"wq{i}")
                nc.sync.dma_start(w[:], wqT_d[128 * i:128 * (i + 1), :])
                wq.append(w)
                w = wgt.tile([128, C], MM_DT, tag=f"wp{i}")
                nc.sync.dma_start(w[:], wpT_d[128 * i:128 * (i + 1), :])
                wp.append(w)
            qb_qk = []
            for j in range(2 * CT):
                t = wgt.tile([128, 1], F32, tag=f"qb{j}")
                nc.sync.dma_start(t[:], qb_d[128 * j:128 * (j + 1)].rearrange("(p a) -> p a", a=1))
                qb_qk.append(t)
            # persistent v^T stores, one per sample; ones-columns are memset
            # once (evacs only write cols 0:64 of each 65-block)
            vt_all = []
            for p in range(BL):
                v = wgt.tile([128, ST * VW], MM_DT, tag=f"vt{p}")
                nc.gpsimd.memset(v[:], 1.0)
                vt_all.append(v)

            # ---------------- phase emitters ----------------
            def gn(b):
                xs = st8[b]["xs"]
                stats = []
                for i in range(CT):
                    bns = scr_p.tile([128, 2, 6], F32, tag="bns", name=f"bns{b}_{i}")
                    xv = xs[i][:].rearrange("p (s f) -> p s f", f=512)
                    for sgi in range(2):
                        nc.vector.bn_stats(bns[:, sgi, :], xv[:, sgi, :])
                    mv = tiny.tile([128, 2], F32, tag="mv", name=f"mv{b}_{i}")
                    nc.vector.bn_aggr(mv[:], bns[:])
                    st = tiny.tile([128, 2], F32, tag="stats", name=f"st{b}_{i}")
                    # sum = L*mean ; sumsq = L*(var + mean^2)
                    nc.vector.tensor_scalar_mul(st[:, 0:1], mv[:, 0:1], float(L))
                    m2 = tiny.tile([128, 2], F32, tag="m2", name=f"m2{b}_{i}")
                    nc.vector.tensor_mul(m2[:, 0:1], mv[:, 0:1], mv[:, 0:1])
                    nc.vector.tensor_add(m2[:, 1:2], mv[:, 1:2], m2[:, 0:1])
                    nc.vector.tensor_scalar_mul(st[:, 1:2], m2[:, 1:2], float(L))
                    stats.append(st)
                pg = ps_p.tile([128, 1024], F32, tag="mm", name=f"pg{b}")
                for i in range(CT):
                    nc.tensor.matmul(pg[0:GROUPS, 0:2], sel[i][:, :], stats[i][:, :],
                                     start=(i == 0), stop=(i == CT - 1))
                # group quantities: mean, E[x2], mean^2, var, std, rstd
                gq = tiny.tile([GROUPS, 8], F32, tag="gq", name=f"gq{b}")
                nc.vector.tensor_scalar_mul(gq[:, 0:1], pg[0:GROUPS, 0:1], INV_N)
                nc.vector.tensor_scalar_mul(gq[:, 1:2], pg[0:GROUPS, 1:2], INV_N)
                nc.vector.tensor_mul(gq[:, 2:3], gq[:, 0:1], gq[:, 0:1])
                nc.vector.tensor_sub(gq[:, 3:4], gq[:, 1:2], gq[:, 2:3])
                nc.scalar.activation(gq[:, 4:5], gq[:, 3:4], ACT.Sqrt,
                                     bias=eps_t[:])
                nc.vector.reciprocal(gq[:, 5:6], gq[:, 4:5])
                nc.vector.tensor_mul(gq[:, 6:7], gq[:, 0:1], gq[:, 5:6])
                # fan out to channels, make per-channel scale/bias
                xn = []
                for i in range(CT):
                    pf = ps_p.tile([128, 1024], F32, tag="mm", name=f"pf{b}_{i}")
                    nc.tensor.matmul(pf[0:128, 0:2], fan[i][:, :], gq[:, 5:7],
                                     start=True, stop=True)
                    scb = tiny.tile([128, 3], F32, tag="scb", name=f"scb{b}_{i}")
                    nc.vector.tensor_mul(scb[:, 0:1], pf[0:128, 0:1], nw[i][:])
                    nc.vector.tensor_mul(scb[:, 1:2], pf[0:128, 1:2], nw[i][:])
                    nc.vector.tensor_sub(scb[:, 2:3], nb[i][:], scb[:, 1:2])
                    t = xn_p.tile([128, L], MM_DT, tag="xn", name=f"xn{b}_{i}")
                    nc.vector.tensor_scalar(t[:], xs[i][:], scb[:, 0:1], scb[:, 2:3],
                                            op0=ALU.mult, op1=ALU.add)
                    xn.append(t)
                    # xs now only feeds the residual: fold b_eff in
                    nc.vector.tensor_scalar_add(xs[i][:], xs[i][:], beff[i][:])
                st8[b]["xn"] = xn

            def vphase(b, half):
                # v^T [s, c] into persistent vt tile, two s-chunks per evac
                xn = st8[b]["xn"]
                vt = vt_all[b]
                vtv = vt[:].rearrange("p (m h x) -> p m h x", h=NH, x=CH + 1)
                for mp in (0, 1) if half == 0 else (2, 3):
                    pv = ps_p.tile([128, 1024], F32, tag="mm", name=f"pv{b}_{mp}")
                    for hf in range(2):
                        m = 2 * mp + hf
                        nsl = slice(512 * hf, 512 * (hf + 1))
                        for i in range(CT):
                            nc.tensor.matmul(
                                pv[:, nsl],
                                xn[i][:, 128 * m:128 * (m + 1)],
                                wq[i][:, 2 * C:3 * C],
                                start=(i == 0), stop=(i == CT - 1))
                    src = pv[:, :].rearrange("p (m h x) -> p m h x", h=NH, x=CH)
                    nc.vector.tensor_copy(vtv[:, 2 * mp:2 * mp + 2, :, 0:CH], src)

            def qkv(b, half):
                xn = st8[b]["xn"]
                qk = st8[b].setdefault("qk", [None] * (2 * CT))
                for j in (range(0, 4) if half == 0 else range(4, 8)):
                    pq = ps_p.tile([128, 1024], F32, tag="mm", name=f"pq{b}_{j}")
                    for n in range(2):
                        nsl = slice(512 * n, 512 * (n + 1))
                        for i in range(CT):
                            nc.tensor.matmul(
                                pq[:, nsl],
                                wq[i][:, 128 * j:128 * (j + 1)],
                                xn[i][:, nsl],
                                start=(i == 0), stop=(i == CT - 1))
                    t = qk_p.tile([128, L], MM_DT, tag="qk", name=f"qk{b}_{j}")
                    nc.vector.tensor_scalar_add(t[:], pq[:, :], qb_qk[j][:])
                    qk[j] = t

            def attn_head(b, h):
                qk = st8[b]["qk"]
                vt = vt_all[b]
                a_tiles = st8[b].setdefault("a", [None] * CT)
                jq = h // 2
                rq = slice(64 * (h % 2), 64 * (h % 2) + 64)
                tp = (64 * (h % 2), 0)
                q_t = qk[jq]
                k_t = qk[CT + jq]
                pa_t = ps_p.tile([CH + 1, 1024], F32, tag="pa",
                                 name=f"pa{b}_{h}")
                prev = None
                for m in range(ST):
                    msl = slice(128 * m, 128 * (m + 1))
                    kq_t = ps_p.tile([128, 1024], F32, tag="mm",
                                     name=f"kq{b}_{h}_{m}")
                    for n in range(2):
                        nsl = slice(512 * n, 512 * (n + 1))
                        nc.tensor.matmul(kq_t[:, nsl],
                                         k_t[rq, msl],
                                         q_t[rq, nsl],
                                         start=True, stop=True,
                                         tile_position=tp)
                    e_t = ew_p.tile([128, L], MM_DT, tag="ew",
                                    name=f"ew{b}_{h}_{m}")
                    nc.scalar.activation(e_t[:], kq_t[:, :], ACT.Exp,
                                         scale=SCALE2)
                    if prev is not None:
                        pm, pe = prev
                        lhs = vt[:, (pm * NH + h) * (CH + 1):
                                 (pm * NH + h + 1) * (CH + 1)]
                        for n in range(2):
                            nsl = slice(512 * n, 512 * (n + 1))
                            nc.tensor.matmul(pa_t[:, nsl], lhs, pe[:, nsl],
                                             start=(pm == 0), stop=False)
                    prev = (m, e_t)
                pm, pe = prev
                lhs = vt[:, (pm * NH + h) * (CH + 1):
                         (pm * NH + h + 1) * (CH + 1)]
                for n in range(2):
                    nsl = slice(512 * n, 512 * (n + 1))
                    nc.tensor.matmul(pa_t[:, nsl], lhs, pe[:, nsl],
                                     start=False, stop=True)
                # normalize: 1/Z (fast approx) -> broadcast -> multiply
                # (reciprocal_approx_fast misreads PSUM in this context —
                # hop through SBUF partition 0 first)
                zt = z_p.tile([1, L], F32, tag="zt", name=f"zt{b}_{h}")
                nc.vector.tensor_copy(zt[:], pa_t[CH:CH + 1, :])
                rz = z_p.tile([1, L], F32, tag="rz", name=f"rz{b}_{h}")
                nc.vector.reciprocal_approx_fast(rz[:], zt[:])
                rzb = zb_p.tile([CH, L], F32, tag="zb", name=f"zb{b}_{h}")
                nc.gpsimd.partition_broadcast(rzb[:], rz[:])
                if h % 2 == 0:
                    a_tiles[h // 2] = a_p.tile([128, L], MM_DT, tag="a",
                                               name=f"a{b}_{h // 2}")
                rows = slice(CH * (h % 2), CH * (h % 2) + CH)
                nc.vector.tensor_mul(a_tiles[h // 2][rows, :],
                                     pa_t[0:CH, :], rzb[:])

            def proj(b):
                xs = st8[b]["xs"]
                a_tiles = st8[b]["a"]
                for j in range(CT):
                    pp = ps_p.tile([128, 1024], F32, tag="mm", name=f"pp{b}_{j}")
                    for n in range(2):
                        nsl = slice(512 * n, 512 * (n + 1))
                        for i in range(CT):
                            nc.tensor.matmul(
                                pp[:, nsl],
                                wp[i][:, 128 * j:128 * (j + 1)],
                                a_tiles[i][:, nsl],
                                start=(i == 0), stop=(i == CT - 1))
                    o_t = out_p.tile([128, L], F32, tag="o", name=f"o{b}_{j}")
                    nc.vector.tensor_add(o_t[:], pp[:, :], xs[j][:])
                    nc.sync.dma_start(out_d[b, 128 * j:128 * (j + 1), :], o_t[:])

            def dump_stage(b, tiles):
                for i in range(CT):
                    nc.sync.dma_start(out_d[b, 128 * i:128 * (i + 1), 0:512],
                                      tiles[i][:].bitcast(F32))

            # ---------------- emission schedule ----------------
            gn(0)
            if STAGE == 1:
                dump_stage(0, st8[0]["xn"])
                load_x(1)
                gn(1)
                dump_stage(1, st8[1]["xn"])
            elif STAGE == 2:
                vphase(0, 0); vphase(0, 1); qkv(0, 0); qkv(0, 1)
                dump_stage(0, st8[0]["qk"][0:CT])
                load_x(1); gn(1)
                vphase(1, 0); vphase(1, 1); qkv(1, 0); qkv(1, 1)
                dump_stage(1, st8[1]["qk"][0:CT])
            else:
                vphase(0, 0); vphase(0, 1)
                qkv(0, 0); qkv(0, 1)
                attn_head(0, 0)
                attn_head(0, 1)
                attn_head(0, 2)
                load_x(1)
                attn_head(0, 3)
                gn(1)
                attn_head(0, 4)
                vphase(1, 0)
                attn_head(0, 5)
                vphase(1, 1)
                attn_head(0, 6)
                qkv(1, 0)
                attn_head(0, 7)
                qkv(1, 1)
                proj(0)
                for h in range(NH):
                    attn_head(1, h)
                proj(1)

    nc.compile()
    return nc


_prog_cache = {}


def _get_program():
    if "prog" not in _prog_cache:
        _prog_cache["prog"] = _build_program()
    return _prog_cache["prog"]


def _host_constants():
    # group selector: sel[i][p, g] = 1 where global group of (tile i, part p)
    # is g;  fan[i][g, p] = same, transposed (for the fan-out matmul lhsT).
    sel = np.zeros((CT, 128, GROUPS), dtype=np.float32)
    fan = np.zeros((CT, GROUPS, 128), dtype=np.float32)
    for i in range(CT):
        for p in range(128):
            g = (128 * i + p) // GS
            sel[i, p, g] = 1.0
            fan[i, g, p] = 1.0
    return sel, fan


def kernel(x, norm_w, norm_b, qkv_w, qkv_b, proj_w, proj_b):
    global LAST_RESULTS
    x = np.ascontiguousarray(np.asarray(x, dtype=np.float32))
    np_mm = mybir.dt.np(MM_DT)
    qkv_w = np.asarray(qkv_w, dtype=np.float32)
    proj_w = np.asarray(proj_w, dtype=np.float32)
    qkv_b = np.ascontiguousarray(np.asarray(qkv_b, dtype=np.float32))
    proj_b = np.ascontiguousarray(np.asarray(proj_b, dtype=np.float32))
    wqkvT = np.ascontiguousarray(qkv_w.T.astype(np_mm))
    wprojT = np.ascontiguousarray(proj_w.T.astype(np_mm))
    # softmax rows sum to 1, so the v-bias contributes exactly
    # proj_w @ v_bias to the proj output; fold it plus proj_b into one
    # per-channel constant added at the residual.
    b_eff = np.ascontiguousarray(
        proj_w @ qkv_b[2 * C:3 * C] + proj_b).astype(np.float32)
    sel, fan = _host_constants()

    xr = x.reshape(B, C, L)
    nc = _get_program()

    common = {
        "wqkvT": wqkvT,
        "wprojT": wprojT,
        "norm_w": np.ascontiguousarray(norm_w, dtype=np.float32),
        "norm_b": np.ascontiguousarray(norm_b, dtype=np.float32),
        "qkv_b": qkv_b,
        "b_eff": b_eff,
        "sel": sel,
        "fan": fan,
    }
    in_maps = []
    for c in range(N_CORES):
        m = dict(common)
        m["x"] = np.ascontiguousarray(xr[BL * c:BL * (c + 1)])
        in_maps.append(m)

    trace = os.environ.get("KERNEL_TRACE", "0") == "1"
    kwargs = {}
    if trace:
        kwargs = dict(trace=True, trace_cores=[0])
    res = run_bass_kernel_spmd(nc, in_maps, core_ids=list(range(N_CORES)),
                               **kwargs)
    LAST_RESULTS = res
    out = np.concatenate([res.results[c]["out"] for c in range(N_CORES)], axis=0)
    return out.reshape(B, C, HH, WW)


# revision 16
# speedup vs baseline: 1.9780x; 1.2214x over previous
"""Trainium2 Bass kernel for nn_AttentionBlock (GroupNorm + 1x1-conv QKV +
multi-head attention + 1x1-conv proj + residual).

Contract: kernel(**inputs) takes the FULL unsharded inputs (numpy) and
returns the FULL output.  Internally shards data-parallel over batch across
8 NeuronCores (2 samples per core).

v3 design notes (baseline 558-619us, v2 348us):
  - Attention runs one head at a time, software-pipelined so ScalarE's exp
    stream is saturated: QK scores land as [128 s, 1024 t] f32 in a 2-bank
    PSUM tile (one [128,1024] exp per (head, s-chunk) amortizes the
    ~350-cycle ACT fixed cost); score tiles round-robin through 2 buffers so
    QK(m+1) overlaps exp(m).
  - AV accumulates [v^T | 1] @ e into a [65, 1024] PSUM tile per head; the
    extra ones-column emits the softmax normalizer Z as row 64.
  - 1/Z via reciprocal_approx_fast (single-pass custom DVE op) after a hop
    through SBUF; broadcast via GpSimd; one [64, 1024] multiply per head.
  - v-bias and proj-bias fold into b_eff = proj_w @ v_bias + proj_b
    (softmax rows sum to 1), added into the residual input on DVE.
  - HAM throttle management (the big v3 win): the PE clock halves (K=4/8)
    after any >3.4us PE-idle window and, once cold, an exp-gated attention
    loop never re-warms.  So: sample 1's GN/V/QKV phases are emitted
    INTERLEAVED into sample 0's attention (the Scalar-bound stretch has PE
    headroom), the V phase precedes QKV (its PSUM evacuations otherwise
    stall attention startup), and x/GN-constant DMAs precede the big weight
    DMAs so compute starts early.
"""

import math
import os

import numpy as np

import concourse.bacc as bacc
import concourse.tile as tile
from concourse import mybir
from concourse.bass_utils import run_bass_kernel_spmd

F32 = mybir.dt.float32
AX = mybir.AxisListType
ALU = mybir.AluOpType
ACT = mybir.ActivationFunctionType

N_CORES = 8
B, C, HH, WW = 16, 512, 32, 32
L = HH * WW            # 1024
BL = B // N_CORES      # batches per core = 2
NH = 8                 # heads
CH = C // NH           # head dim = 64
GROUPS = 32
GS = C // GROUPS       # channels per group = 16
EPS = 1e-5
SCALE2 = 1.0 / math.sqrt(CH)   # combined q*k scale, folded into exp
CT = C // 128          # channel tiles = 4
ST = L // 128          # s-chunks = 8
INV_N = 1.0 / (GS * L)         # 1/16384 for group mean

MM_DT = mybir.dt.bfloat16
# debug bisection: 1=gn, 2=+qkv, 4=full, 5=dump Z
STAGE = int(os.environ.get("KERNEL_STAGE", "4"))

LAST_RESULTS = None  # test harness can read exec_time_ns from here


def _build_program():
    nc = bacc.Bacc("TRN2", target_bir_lowering=False, debug=False,
                   num_devices=N_CORES)

    x_d = nc.dram_tensor("x", [BL, C, L], F32, kind="ExternalInput").ap()
    out_d = nc.dram_tensor("out", [BL, C, L], F32, kind="ExternalOutput").ap()
    wqT_d = nc.dram_tensor("wqkvT", [C, 3 * C], MM_DT, kind="ExternalInput").ap()
    wpT_d = nc.dram_tensor("wprojT", [C, C], MM_DT, kind="ExternalInput").ap()
    nw_d = nc.dram_tensor("norm_w", [C], F32, kind="ExternalInput").ap()
    nb_d = nc.dram_tensor("norm_b", [C], F32, kind="ExternalInput").ap()
    qb_d = nc.dram_tensor("qkv_b", [3 * C], F32, kind="ExternalInput").ap()
    beff_d = nc.dram_tensor("b_eff", [C], F32, kind="ExternalInput").ap()
    sel_d = nc.dram_tensor("sel", [CT, 128, GROUPS], F32, kind="ExternalInput").ap()
    fan_d = nc.dram_tensor("fan", [CT, GROUPS, 128], F32, kind="ExternalInput").ap()

    VW = NH * (CH + 1)         # 520: per-s-chunk v^T row width (8 heads x 65)

    with tile.TileContext(nc) as tc:
        with (
            tc.tile_pool(name="wgt", bufs=1) as wgt,          # persistent
            tc.tile_pool(name="xs", bufs=2 * CT) as xs_p,     # raw x tiles
            tc.tile_pool(name="xn", bufs=2 * CT) as xn_p,     # normalized x
            tc.tile_pool(name="qk", bufs=4 * CT) as qk_p,     # q,k both samples
            tc.tile_pool(name="ew", bufs=4) as ew_p,          # exp(wT) chunks
            tc.tile_pool(name="apool", bufs=2 * CT) as a_p,   # attention out
            tc.tile_pool(name="zz", bufs=2) as z_p,           # Z rows / 1/Z
            tc.tile_pool(name="zb", bufs=2) as zb_p,          # 1/Z broadcast
            tc.tile_pool(name="outs", bufs=2) as out_p,       # residual out
            tc.tile_pool(name="tiny", bufs=8) as tiny,        # gn stats etc.
            tc.tile_pool(name="scr", bufs=2) as scr_p,        # bn stats
            tc.tile_pool(name="ps", bufs=2, space="PSUM") as ps_p,  # shared
        ):
            # ---------------- x(b=0) + GN constants first ----------------
            st8 = {}   # per-sample dicts of live tiles
            for b in range(BL):
                st8[b] = {}

            def load_x(b):
                xs = []
                for i in range(CT):
                    t = xs_p.tile([128, L], F32, tag="xs", name=f"xs{b}_{i}")
                    nc.sync.dma_start(t[:], x_d[b, 128 * i:128 * (i + 1), :])
                    xs.append(t)
                st8[b]["xs"] = xs

            load_x(0)

            sel = []
            fan = []
            nw = []
            nb = []
            beff = []
            for i in range(CT):
                s = wgt.tile([128, GROUPS], F32, tag=f"sel{i}")
                nc.sync.dma_start(s[:], sel_d[i, :, :])
                sel.append(s)
                f = wgt.tile([GROUPS, 128], F32, tag=f"fan{i}")
                nc.sync.dma_start(f[:], fan_d[i, :, :])
                fan.append(f)
                t = wgt.tile([128, 1], F32, tag=f"nw{i}")
                nc.sync.dma_start(t[:], nw_d[128 * i:128 * (i + 1)].rearrange("(p a) -> p a", a=1))
                nw.append(t)
                t = wgt.tile([128, 1], F32, tag=f"nb{i}")
                nc.sync.dma_start(t[:], nb_d[128 * i:128 * (i + 1)].rearrange("(p a) -> p a", a=1))
                nb.append(t)
                t = wgt.tile([128, 1], F32, tag=f"beff{i}")
                nc.sync.dma_start(t[:], beff_d[128 * i:128 * (i + 1)].rearrange("(p a) -> p a", a=1))
                beff.append(t)
            eps_t = wgt.tile([GROUPS, 1], F32, tag="eps")
            nc.gpsimd.memset(eps_t[:], EPS)

            # ---------------- big weights ----------------
            wq = []   # qkv_w^T tiles [128 c', 1536 o]
            wp = []   # proj_w^T tiles [128 c', 512 o]
            for i in range(CT):
                w = wgt.tile([128, 3 * C], MM_DT, tag=f"wq{i}")
                nc.sync.dma_start(w[:], wqT_d[128 * i:128 * (i + 1), :])
                wq.append(w)
                w = wgt.tile([128, C], MM_DT, tag=f"wp{i}")
                nc.sync.dma_start(w[:], wpT_d[128 * i:128 * (i + 1), :])
                wp.append(w)
            qb_qk = []
            for j in range(2 * CT):
                t = wgt.tile([128, 1], F32, tag=f"qb{j}")
                nc.sync.dma_start(t[:], qb_d[128 * j:128 * (j + 1)].rearrange("(p a) -> p a", a=1))
                qb_qk.append(t)
            # persistent v^T stores, one per sample; ones-columns are memset
            # once (evacs only write cols 0:64 of each 65-block)
            vt_all = []
            for p in range(BL):
                v = wgt.tile([128, ST * VW], MM_DT, tag=f"vt{p}")
                nc.gpsimd.memset(v[:], 1.0)
                vt_all.append(v)

            # ---------------- phase emitters ----------------
            def gn(b):
                xs = st8[b]["xs"]
                stats = []
                for i in range(CT):
                    bns = scr_p.tile([128, 2, 6], F32, tag="bns", name=f"bns{b}_{i}")
                    xv = xs[i][:].rearrange("p (s f) -> p s f", f=512)
                    for sgi in range(2):
                        nc.vector.bn_stats(bns[:, sgi, :], xv[:, sgi, :])
                    mv = tiny.tile([128, 2], F32, tag="mv", name=f"mv{b}_{i}")
                    nc.vector.bn_aggr(mv[:], bns[:])
                    st = tiny.tile([128, 2], F32, tag="stats", name=f"st{b}_{i}")
                    # sum = L*mean ; sumsq = L*(var + mean^2)
                    nc.vector.tensor_scalar_mul(st[:, 0:1], mv[:, 0:1], float(L))
                    m2 = tiny.tile([128, 2], F32, tag="m2", name=f"m2{b}_{i}")
                    nc.vector.tensor_mul(m2[:, 0:1], mv[:, 0:1], mv[:, 0:1])
                    nc.vector.tensor_add(m2[:, 1:2], mv[:, 1:2], m2[:, 0:1])
                    nc.vector.tensor_scalar_mul(st[:, 1:2], m2[:, 1:2], float(L))
                    stats.append(st)
                pg = ps_p.tile([128, 1024], F32, tag="mm", name=f"pg{b}")
                for i in range(CT):
                    nc.tensor.matmul(pg[0:GROUPS, 0:2], sel[i][:, :], stats[i][:, :],
                                     start=(i == 0), stop=(i == CT - 1))
                # group quantities: mean, E[x2], mean^2, var, std, rstd
                gq = tiny.tile([GROUPS, 8], F32, tag="gq", name=f"gq{b}")
                nc.vector.tensor_scalar_mul(gq[:, 0:1], pg[0:GROUPS, 0:1], INV_N)
                nc.vector.tensor_scalar_mul(gq[:, 1:2], pg[0:GROUPS, 1:2], INV_N)
                nc.vector.tensor_mul(gq[:, 2:3], gq[:, 0:1], gq[:, 0:1])
                nc.vector.tensor_sub(gq[:, 3:4], gq[:, 1:2], gq[:, 2:3])
                nc.scalar.activation(gq[:, 4:5], gq[:, 3:4], ACT.Sqrt,
                                     bias=eps_t[:])
                nc.vector.reciprocal(gq[:, 5:6], gq[:, 4:5])
                nc.vector.tensor_mul(gq[:, 6:7], gq[:, 0:1], gq[:, 5:6])
                # fan out to channels, make per-channel scale/bias
                xn = []
                for i in range(CT):
                    pf = ps_p.tile([128, 1024], F32, tag="mm", name=f"pf{b}_{i}")
                    nc.tensor.matmul(pf[0:128, 0:2], fan[i][:, :], gq[:, 5:7],
                                     start=True, stop=True)
                    scb = tiny.tile([128, 3], F32, tag="scb", name=f"scb{b}_{i}")
                    nc.vector.tensor_mul(scb[:, 0:1], pf[0:128, 0:1], nw[i][:])
                    nc.vector.tensor_mul(scb[:, 1:2], pf[0:128, 1:2], nw[i][:])
                    nc.vector.tensor_sub(scb[:, 2:3], nb[i][:], scb[:, 1:2])
                    t = xn_p.tile([128, L], MM_DT, tag="xn", name=f"xn{b}_{i}")
                    nc.vector.tensor_scalar(t[:], xs[i][:], scb[:, 0:1], scb[:, 2:3],
                                            op0=ALU.mult, op1=ALU.add)
                    xn.append(t)
                    # xs now only feeds the residual: fold b_eff in
                    nc.vector.tensor_scalar_add(xs[i][:], xs[i][:], beff[i][:])
                st8[b]["xn"] = xn

            def vphase(b, half):
                # v^T [s, c] into persistent vt tile, two s-chunks per evac
                xn = st8[b]["xn"]
                vt = vt_all[b]
                vtv = vt[:].rearrange("p (m h x) -> p m h x", h=NH, x=CH + 1)
                for mp in (0, 1) if half == 0 else (2, 3):
                    pv = ps_p.tile([128, 1024], F32, tag="mm", name=f"pv{b}_{mp}")
                    for hf in range(2):
                        m = 2 * mp + hf
                        nsl = slice(512 * hf, 512 * (hf + 1))
                        for i in range(CT):
                            nc.tensor.matmul(
                                pv[:, nsl],
                                xn[i][:, 128 * m:128 * (m + 1)],
                                wq[i][:, 2 * C:3 * C],
                                start=(i == 0), stop=(i == CT - 1))
                    src = pv[:, :].rearrange("p (m h x) -> p m h x", h=NH, x=CH)
                    nc.vector.tensor_copy(vtv[:, 2 * mp:2 * mp + 2, :, 0:CH], src)

            def qkv(b, half):
                xn = st8[b]["xn"]
                qk = st8[b].setdefault("qk", [None] * (2 * CT))
                for j in (range(0, 4) if half == 0 else range(4, 8)):
                    pq = ps_p.tile([128, 1024], F32, tag="mm", name=f"pq{b}_{j}")
                    for n in range(2):
                        nsl = slice(512 * n, 512 * (n + 1))
                        for i in range(CT):
                            nc.tensor.matmul(
                                pq[:, nsl],
                                wq[i][:, 128 * j:128 * (j + 1)],
                                xn[i][:, nsl],
                                start=(i == 0), stop=(i == CT - 1))
                    t = qk_p.tile([128, L], MM_DT, tag="qk", name=f"qk{b}_{j}")
                    nc.vector.tensor_scalar_add(t[:], pq[:, :], qb_qk[j][:])
                    qk[j] = t

            def attn_head(b, h, insert_after_m=None):
                qk = st8[b]["qk"]
                vt = vt_all[b]
                a_tiles = st8[b].setdefault("a", [None] * CT)
                jq = h // 2
                rq = slice(64 * (h % 2), 64 * (h % 2) + 64)
                tp = (64 * (h % 2), 0)
                q_t = qk[jq]
                k_t = qk[CT + jq]
                pa_t = ps_p.tile([CH + 1, 1024], F32, tag="pa",
                                 name=f"pa{b}_{h}")
                prev = None
                for m in range(ST):
                    msl = slice(128 * m, 128 * (m + 1))
                    kq_t = ps_p.tile([128, 1024], F32, tag="mm",
                                     name=f"kq{b}_{h}_{m}")
                    for n in range(2):
                        nsl = slice(512 * n, 512 * (n + 1))
                        nc.tensor.matmul(kq_t[:, nsl],
                                         k_t[rq, msl],
                                         q_t[rq, nsl],
                                         start=True, stop=True,
                                         tile_position=tp)
                    e_t = ew_p.tile([128, L], MM_DT, tag="ew",
                                    name=f"ew{b}_{h}_{m}")
                    nc.scalar.activation(e_t[:], kq_t[:, :], ACT.Exp,
                                         scale=SCALE2)
                    if prev is not None:
                        pm, pe = prev
                        lhs = vt[:, (pm * NH + h) * (CH + 1):
                                 (pm * NH + h + 1) * (CH + 1)]
                        for n in range(2):
                            nsl = slice(512 * n, 512 * (n + 1))
                            nc.tensor.matmul(pa_t[:, nsl], lhs, pe[:, nsl],
                                             start=(pm == 0), stop=False)
                    prev = (m, e_t)
                    if insert_after_m and m in insert_after_m:
                        insert_after_m[m]()
                pm, pe = prev
                lhs = vt[:, (pm * NH + h) * (CH + 1):
                         (pm * NH + h + 1) * (CH + 1)]
                for n in range(2):
                    nsl = slice(512 * n, 512 * (n + 1))
                    nc.tensor.matmul(pa_t[:, nsl], lhs, pe[:, nsl],
                                     start=False, stop=True)
                # normalize: 1/Z (fast approx) -> broadcast -> multiply
                # (reciprocal_approx_fast misreads PSUM in this context —
                # hop through SBUF partition 0 first)
                zt = z_p.tile([1, L], F32, tag="zt", name=f"zt{b}_{h}")
                nc.vector.tensor_copy(zt[:], pa_t[CH:CH + 1, :])
                rz = z_p.tile([1, L], F32, tag="rz", name=f"rz{b}_{h}")
                nc.vector.reciprocal_approx_fast(rz[:], zt[:])
                rzb = zb_p.tile([CH, L], F32, tag="zb", name=f"zb{b}_{h}")
                nc.gpsimd.partition_broadcast(rzb[:], rz[:])
                if h % 2 == 0:
                    a_tiles[h // 2] = a_p.tile([128, L], MM_DT, tag="a",
                                               name=f"a{b}_{h // 2}")
                rows = slice(CH * (h % 2), CH * (h % 2) + CH)
                nc.vector.tensor_mul(a_tiles[h // 2][rows, :],
                                     pa_t[0:CH, :], rzb[:])

            def proj(b, js=range(CT)):
                xs = st8[b]["xs"]
                a_tiles = st8[b]["a"]
                for j in js:
                    pp = ps_p.tile([128, 1024], F32, tag="mm", name=f"pp{b}_{j}")
                    for n in range(2):
                        nsl = slice(512 * n, 512 * (n + 1))
                        for i in range(CT):
                            nc.tensor.matmul(
                                pp[:, nsl],
                                wp[i][:, 128 * j:128 * (j + 1)],
                                a_tiles[i][:, nsl],
                                start=(i == 0), stop=(i == CT - 1))
                    o_t = out_p.tile([128, L], F32, tag="o", name=f"o{b}_{j}")
                    nc.vector.tensor_add(o_t[:], pp[:, :], xs[j][:])
                    nc.sync.dma_start(out_d[b, 128 * j:128 * (j + 1), :], o_t[:])

            def dump_stage(b, tiles):
                for i in range(CT):
                    nc.sync.dma_start(out_d[b, 128 * i:128 * (i + 1), 0:512],
                                      tiles[i][:].bitcast(F32))

            # ---------------- emission schedule ----------------
            gn(0)
            if STAGE == 1:
                dump_stage(0, st8[0]["xn"])
                load_x(1)
                gn(1)
                dump_stage(1, st8[1]["xn"])
            elif STAGE == 2:
                vphase(0, 0); vphase(0, 1); qkv(0, 0); qkv(0, 1)
                dump_stage(0, st8[0]["qk"][0:CT])
                load_x(1); gn(1)
                vphase(1, 0); vphase(1, 1); qkv(1, 0); qkv(1, 1)
                dump_stage(1, st8[1]["qk"][0:CT])
            else:
                # Sequential dense blocks; every attention block is entered
                # with the PE warm (preceded by full-array matmul phases) and
                # is never PE-idle >~3.4us inside, so the HAM clock gate
                # stays at 8/8.  Both GN phases run up-front (all Sqrt
                # activations precede the first Exp: one table load each).
                load_x(1)
                gn(1)
                vphase(0, 0); vphase(0, 1)
                qkv(0, 0); qkv(0, 1)
                for h in range(NH):
                    attn_head(0, h)
                vphase(1, 0); vphase(1, 1)
                qkv(1, 0); qkv(1, 1)
                proj(0, js=(0, 1, 2))
                # densify the A(1) entry ramp with proj(0)'s last block
                attn_head(1, 0, insert_after_m={1: (lambda: proj(0, js=(3,)))})
                for h in range(1, NH):
                    attn_head(1, h)
                proj(1)

    nc.compile()
    return nc


_prog_cache = {}


def _get_program():
    if "prog" not in _prog_cache:
        _prog_cache["prog"] = _build_program()
    return _prog_cache["prog"]


def _host_constants():
    # group selector: sel[i][p, g] = 1 where global group of (tile i, part p)
    # is g;  fan[i][g, p] = same, transposed (for the fan-out matmul lhsT).
    sel = np.zeros((CT, 128, GROUPS), dtype=np.float32)
    fan = np.zeros((CT, GROUPS, 128), dtype=np.float32)
    for i in range(CT):
        for p in range(128):
            g = (128 * i + p) // GS
            sel[i, p, g] = 1.0
            fan[i, g, p] = 1.0
    return sel, fan


def kernel(x, norm_w, norm_b, qkv_w, qkv_b, proj_w, proj_b):
    global LAST_RESULTS
    x = np.ascontiguousarray(np.asarray(x, dtype=np.float32))
    np_mm = mybir.dt.np(MM_DT)
    qkv_w = np.asarray(qkv_w, dtype=np.float32)
    proj_w = np.asarray(proj_w, dtype=np.float32)
    qkv_b = np.ascontiguousarray(np.asarray(qkv_b, dtype=np.float32))
    proj_b = np.ascontiguousarray(np.asarray(proj_b, dtype=np.float32))
    wqkvT = np.ascontiguousarray(qkv_w.T.astype(np_mm))
    wprojT = np.ascontiguousarray(proj_w.T.astype(np_mm))
    # softmax rows sum to 1, so the v-bias contributes exactly
    # proj_w @ v_bias to the proj output; fold it plus proj_b into one
    # per-channel constant added at the residual.
    b_eff = np.ascontiguousarray(
        proj_w @ qkv_b[2 * C:3 * C] + proj_b).astype(np.float32)
    sel, fan = _host_constants()

    xr = x.reshape(B, C, L)
    nc = _get_program()

    common = {
        "wqkvT": wqkvT,
        "wprojT": wprojT,
        "norm_w": np.ascontiguousarray(norm_w, dtype=np.float32),
        "norm_b": np.ascontiguousarray(norm_b, dtype=np.float32),
        "qkv_b": qkv_b,
        "b_eff": b_eff,
        "sel": sel,
        "fan": fan,
    }
    in_maps = []
    for c in range(N_CORES):
        m = dict(common)
        m["x"] = np.ascontiguousarray(xr[BL * c:BL * (c + 1)])
        in_maps.append(m)

    trace = os.environ.get("KERNEL_TRACE", "0") == "1"
    kwargs = {}
    if trace:
        kwargs = dict(trace=True, trace_cores=[0])
    res = run_bass_kernel_spmd(nc, in_maps, core_ids=list(range(N_CORES)),
                               **kwargs)
    LAST_RESULTS = res
    out = np.concatenate([res.results[c]["out"] for c in range(N_CORES)], axis=0)
    return out.reshape(B, C, HH, WW)
